# revision 13
# baseline (speedup 1.0000x reference)
"""Causal self-attention kernel for 8 Trainium2 NeuronCores.

Sharding: core c -> (batch b = c // 2, head-group g = c % 2).
Each core computes attention for its batch over its 8 heads and a partial
output projection; the host sums the two head-group partials per batch and
adds b_proj.

v4: single fused pipeline.  The attention stream (S matmul pairs -> ACT exp
-> PV matmul pairs) is the backbone; all dense matmul work (qkv projection,
v tiles, output projection) is interleaved into it from a queue so the PE
never waits on the scalar engine.  x is pre-transposed and pre-cast to bf16
on the host (xT) and DMA'd directly into SBUF, as are all weights (one
batched strided DMA each, on the scalar HWDGE queue, concurrent with the
sync queue).  Causal masking zeroes the exp output on gpsimd
(affine_select).  1/l = exp(-ln(l)) on ACT - same table set as exp, no
table switches - computed per (i-chunk, head-pair) so normalization and
projection pipeline tightly behind attention.

Reference shapes: x [4, 2048, 1024], W_attn [1024, 3072], b_attn [3072],
W_proj [1024, 1024], b_proj [1024]; NH=16, HD=64.
"""

import ml_dtypes
import numpy as np

import bass_rust
import concourse.bass as bass
import concourse.mybir as mybir
import concourse.tile as tile
from concourse.bass_utils import run_bass_kernel_spmd

DT = mybir.dt
AF = mybir.ActivationFunctionType
ALU = mybir.AluOpType

P = 128
T = 2048          # sequence length
CIN = 1024        # input channels
CL = 512          # local channels (8 heads x 64)
NHL = 8           # local heads
HD = 64
KT = CIN // P     # 8 contraction tiles for qkv
TT = T // P       # 16 t-tiles
IC = T // 512     # 4 i-chunks of 512
COUT = 1024       # proj output channels
SCALE = 1.0 / 8.0  # 1/sqrt(HD), folded into wq/bq on host
LAG = 2


class PatchedTileContext(tile.TileContext):
    """Work around walrus's 1-sync-wait-per-Drain limit: split the final
    drain's waits across one Drain instruction per proc."""

    def _drain_and_barrier(self, tick_clock, wait_clock):
        ScopedClock = bass_rust.ScopedClock
        VectorClock = bass_rust.VectorClock
        ticks = eval(repr(tick_clock.global_clock).replace("VectorClock(", "").rstrip(")"))
        for p, t in [(p, t) for p, t in enumerate(ticks) if t > 0]:
            part = [0] * len(ticks)
            part[p] = t
            d = self.nc.sync.drain()
            wait_clock.add_sem_waits(d.ins, ScopedClock({None: VectorClock(part)}))
        self.nc.all_engine_barrier()
        popped = self.nc._tile_sem_poison_stack.pop()
        assert popped is self._sem_poison
        self.nc.clear_and_free_semaphores(list(self.sems.allocated().values()))
        self.nc.all_engine_barrier()


_MAX_WAITS = {}
_MAX_WAITS_DEFAULT = 1


def split_multi_waits(nc):
    for fn in nc.m.functions:
        for blk in fn.blocks:
            insts = blk.instructions
            out = []
            for inst in insts:
                si = getattr(inst, "sync_info", None)
                waits = list(si.on_wait) if si is not None and si.on_wait else []
                cap = _MAX_WAITS.get(str(inst.opcode), _MAX_WAITS_DEFAULT)
                if len(waits) > cap:
                    extra, keep = waits[:-cap], waits[-cap:]
                    for k, w in enumerate(extra):
                        nn = mybir.InstNoOp(name=f"{inst.name}-w{k}", ins=[], outs=[])
                        nn.engine = inst.engine
                        nn.sync_info = bass_rust.SyncInfo(on_wait=[w], on_update=[])
                        out.append(nn)
                    inst.sync_info = bass_rust.SyncInfo(
                        on_wait=keep, on_update=list(si.on_update or []))
                out.append(inst)
            blk.instructions = out


def build_program(split_waits=True):
    nc = bass.Bass()
    xT_d = nc.dram_tensor("xT", [CIN, T], DT.bfloat16, kind="ExternalInput")
    wqk_d = nc.dram_tensor("wqk", [CIN, 2 * CL], DT.bfloat16, kind="ExternalInput")
    wv_d = nc.dram_tensor("wv", [CIN, CL], DT.bfloat16, kind="ExternalInput")
    bqk_d = nc.dram_tensor("bqk", [2 * CL], DT.float32, kind="ExternalInput")
    bv_d = nc.dram_tensor("bv", [CL], DT.float32, kind="ExternalInput")
    wp_d = nc.dram_tensor("wp", [CL, COUT], DT.bfloat16, kind="ExternalInput")
    out_d = nc.dram_tensor("out", [T, COUT], DT.float32, kind="ExternalOutput")

    xT_r = xT_d.rearrange("(ko p) t -> p ko t", p=P)     # [128, 8, 2048]
    wqk_r = wqk_d.rearrange("(ko p) n -> p ko n", p=P)   # [128, 8, 1024]
    wv_r = wv_d.rearrange("(ko p) n -> p ko n", p=P)     # [128, 8, 512]
    wp_r = wp_d.rearrange("(ko p) n -> p ko n", p=P)     # [128, 4, 1024]
    out_r = out_d.rearrange("(tt p) c -> p tt c", p=P)   # [128, 16, 1024]

    with PatchedTileContext(nc) as tc:
        with (
            tc.tile_pool(name="const", bufs=1) as const,
            tc.tile_pool(name="big", bufs=1) as big,
            tc.tile_pool(name="pt", bufs=8) as pt_pool,
            tc.tile_pool(name="rr", bufs=2) as rr_pool,
            tc.tile_pool(name="outp", bufs=3) as outp,
            tc.tile_pool(name="ps_s", bufs=2, space="PSUM") as ps_s,
            tc.tile_pool(name="ps_y", bufs=2, space="PSUM") as ps_y,
            tc.tile_pool(name="ps_d", bufs=2, space="PSUM") as ps_d,
        ):
            # ---- constants ----
            ones1 = const.tile([65, P], DT.bfloat16, tag="ones1")
            nc.gpsimd.memset(ones1[:], 1.0)
            bqk_sb = const.tile([P, 2 * CL // P], DT.float32, tag="bqk")
            nc.sync.dma_start(bqk_sb[:], bqk_d.rearrange("(mt p) -> p mt", p=P))
            bv_sb = const.tile([P, NHL // 2], DT.float32, tag="bv")
            nc.sync.dma_start(
                bv_sb[:], bv_d.rearrange("(hp t p) -> (t p) hp", t=2, p=HD))

            # ---- persistent SBUF tensors ----
            xT_bf = big.tile([P, KT, T], DT.bfloat16, tag="xT")
            qkT_bf = big.tile([P, KT, T], DT.bfloat16, tag="qkT")
            v_sb = big.tile([P, TT, NHL, HD + 1], DT.bfloat16, tag="v_sb")
            yT_bf = big.tile([P, CL // P, T], DT.bfloat16, tag="yT")
            wqk_bf = big.tile([P, KT, 2 * CL], DT.bfloat16, tag="wqk_bf")
            wv_bf = big.tile([P, KT, CL], DT.bfloat16, tag="wv_bf")
            wp_bf = big.tile([P, CL // P, COUT], DT.bfloat16, tag="wp_bf")
            # l rows: head 0 at partition 0, head 1 at partition 64;
            # chunk 4*ic + hp
            l_buf = big.tile([65, 4 * IC, 512], DT.bfloat16, tag="l_buf")

            nc.gpsimd.memset(v_sb[:, :, :, HD], 1.0)

            # ---- head DMAs: weights on the scalar HWDGE queue,
            # xT chunks on the sync queue (concurrent) ----
            nc.scalar.dma_start(wqk_bf[:], wqk_r[:])
            nc.sync.dma_start(xT_bf[:, :, 0:512], xT_r[:, :, 0:512])
            nc.scalar.dma_start(wv_bf[:], wv_r[:])
            nc.scalar.dma_start(wp_bf[:], wp_r[:])
            for nic in range(1, 4):
                nc.sync.dma_start(
                    xT_bf[:, :, nic * 512:(nic + 1) * 512],
                    xT_r[:, :, nic * 512:(nic + 1) * 512])

            # ---- pipeline unit emitters ----
            def qkT_unit(nic, mi):
                pq = ps_d.tile([P, 512], DT.float32, tag="d", name=f"qk{nic}_{mi}")
                for ki in range(KT):
                    nc.tensor.matmul(
                        pq[:],
                        wqk_bf[:, ki, mi * P:(mi + 1) * P],
                        xT_bf[:, ki, nic * 512:(nic + 1) * 512],
                        start=(ki == 0), stop=(ki == KT - 1),
                    )
                nc.vector.tensor_scalar_add(
                    qkT_bf[:, mi, nic * 512:(nic + 1) * 512],
                    pq[:], bqk_sb[:, mi:mi + 1])

            def v_unit(tt):
                pv = ps_d.tile([P, 512], DT.float32, tag="d", name=f"v{tt}")
                for ki in range(KT):
                    nc.tensor.matmul(
                        pv[:],
                        xT_bf[:, ki, tt * P:(tt + 1) * P],
                        wv_bf[:, ki, :],
                        start=(ki == 0), stop=(ki == KT - 1),
                    )
                nc.vector.tensor_copy(
                    v_sb[:, tt, :, 0:HD],
                    pv[:].rearrange("p (h e) -> p h e", h=NHL),
                )

            def norm_unit(ic, hp):
                """yT[:, hp, ic-chunk] = z * exp(-ln(l)) + bv."""
                c = 4 * ic + hp
                rr1 = rr_pool.tile([65, 512], DT.float32, tag="rr1", name=f"ln{c}")
                nc.scalar.activation(rr1[:], l_buf[:, c, :], AF.Ln)
                rr2 = rr_pool.tile([65, 512], DT.bfloat16, tag="rr2", name=f"rx{c}")
                nc.scalar.activation(rr2[:], rr1[:], AF.Exp, scale=-1.0)
                pb = ps_d.tile([P, 512], DT.float32, tag="d", name=f"nb{c}")
                nc.tensor.matmul(
                    pb[0:HD, :], ones1[0:1, 0:HD], rr2[0:1, :],
                    start=True, stop=True)
                nc.tensor.matmul(
                    pb[HD:P, :], ones1[64:65, 0:HD], rr2[64:65, :],
                    start=True, stop=True, tile_position=(64, HD))
                ysl = yT_bf[:, hp, ic * 512:(ic + 1) * 512]
                nc.vector.tensor_mul(ysl, ysl, pb[:])
                nc.vector.tensor_scalar_add(ysl, ysl, bv_sb[:, hp:hp + 1])

            def proj_unit(tt, oc):
                pp = ps_d.tile([P, 512], DT.float32, tag="d", name=f"pj{tt}_{oc}")
                for ci in range(CL // P):
                    nc.tensor.matmul(
                        pp[:],
                        yT_bf[:, ci, tt * P:(tt + 1) * P],
                        wp_bf[:, ci, oc * 512:(oc + 1) * 512],
                        start=(ci == 0), stop=(ci == CL // P - 1),
                    )
                ot = outp.tile([P, 512], DT.float32, tag="ot", name=f"ot{tt}_{oc}")
                nc.vector.tensor_copy(ot[:], pp[:])
                nc.sync.dma_start(out_r[:, tt, oc * 512:(oc + 1) * 512], ot[:])

            # ---- dense work queue ----
            done_units = set()
            queue = []
            pop_idx = [0]

            def emit_unit(idx):
                key, fn = queue[idx]
                fn()
                done_units.add(key)

            def pop_one():
                while pop_idx[0] < len(queue):
                    i = pop_idx[0]
                    pop_idx[0] += 1
                    if queue[i][0] not in done_units:
                        emit_unit(i)
                        return True
                return False

            def pop_until(*keys):
                need = [k for k in keys if k not in done_units]
                for k in need:
                    assert any(q[0] == k for q in queue), f"missing unit {k}"
                while need:
                    if not pop_one():
                        raise AssertionError(f"queue exhausted, need {need}")
                    need = [k for k in keys if k not in done_units]

            def U(key, fn, *args):
                queue.append((key, (lambda f=fn, a=args: f(*a))))

            def U_next(key, fn, *args):
                queue.insert(pop_idx[0], (key, (lambda f=fn, a=args: f(*a))))

            # head compute
            qkT_unit(0, 0)
            qkT_unit(0, 4)
            done_units.update({("qkT", 0, 0), ("qkT", 0, 4)})

            U(("qkT", 0, 1), qkT_unit, 0, 1)
            U(("qkT", 0, 5), qkT_unit, 0, 5)
            for tt in range(4):
                U(("v", tt), v_unit, tt)
            U(("qkT", 0, 2), qkT_unit, 0, 2)
            U(("qkT", 0, 6), qkT_unit, 0, 6)
            U(("qkT", 0, 3), qkT_unit, 0, 3)
            U(("qkT", 0, 7), qkT_unit, 0, 7)
            for b in range(1, 4):
                for j, mi in enumerate([0, 4, 1, 5, 2, 6, 3, 7]):
                    U(("qkT", b, mi), qkT_unit, b, mi)
                    if j == 3:
                        for tt in range(4 * b, 4 * b + 4):
                            U(("v", tt), v_unit, tt)

            # paced pops per stripe, by i-chunk
            pace = {0: 2, 1: 2, 2: 2, 3: 6}

            # ---- attention ----
            for ic in range(IC):
                jt_max = 4 * ic + 3
                for hp in range(NHL // 2):
                    hA, hB = 2 * hp, 2 * hp + 1
                    qt, kt_i = hp, 4 + hp
                    pop_until(
                        ("qkT", ic, hp),
                        *[("qkT", n, 4 + hp) for n in range(ic + 1)],
                    )
                    budget = [pace[ic]]
                    pyA = ps_y.tile([HD + 1, 512], DT.float32, tag="y", name="pyA")
                    pyB = ps_y.tile([HD + 1, 512], DT.float32, tag="y", name="pyB")
                    pts = []

                    def emit_pv(jt):
                        pop_until(("v", jt))
                        d = jt - 4 * ic
                        off = 128 * d if d > 0 else 0
                        pt = pts[jt]
                        nc.tensor.matmul(
                            pyA[:, off:512], v_sb[:, jt, hA, :],
                            pt[:, off:512],
                            start=(jt == 0), stop=(jt == jt_max),
                            skip_group_check=True)
                        nc.tensor.matmul(
                            pyB[:, off:512], v_sb[:, jt, hB, :],
                            pt[:, 512 + off:1024],
                            start=(jt == 0), stop=(jt == jt_max),
                            skip_group_check=True)

                    for jt in range(jt_max + 1):
                        d = jt - 4 * ic
                        off = 128 * d if d > 0 else 0
                        ps = ps_s.tile([P, 1024], DT.float32, tag="s", name="st")
                        isl = slice(ic * 512 + off, (ic + 1) * 512)
                        nc.tensor.matmul(
                            ps[:, off:512],
                            qkT_bf[0:HD, kt_i, jt * P:(jt + 1) * P],
                            qkT_bf[0:HD, qt, isl],
                            start=True, stop=True)
                        nc.tensor.matmul(
                            ps[:, 512 + off:1024],
                            qkT_bf[HD:P, kt_i, jt * P:(jt + 1) * P],
                            qkT_bf[HD:P, qt, isl],
                            start=True, stop=True)
                        pt = pt_pool.tile([P, 1024], DT.bfloat16, tag="pt")
                        if d >= 0:
                            ps2 = ps[:].rearrange("p (g x) -> p g x", g=2)
                            pt2 = pt[:].rearrange("p (g x) -> p g x", g=2)
                            nc.scalar.activation(
                                pt2[:, :, off:512], ps2[:, :, off:512], AF.Exp)
                            nc.gpsimd.affine_select(
                                out=pt2[:, :, off:off + P],
                                in_=pt2[:, :, off:off + P],
                                compare_op=ALU.is_ge,
                                fill=0.0,
                                base=0,
                                pattern=[[0, 2], [1, P]],
                                channel_multiplier=-1,
                            )
                        else:
                            nc.scalar.activation(pt[:], ps[:], AF.Exp)
                        pts.append(pt)
                        if jt >= LAG:
                            emit_pv(jt - LAG)
                            if jt % 2 == 0 and budget[0] > 0:
                                if pop_one():
                                    budget[0] -= 1
                    for jt in range(max(0, jt_max + 1 - LAG), jt_max + 1):
                        emit_pv(jt)
                    # stash unnormalized z and the l rows
                    nc.vector.tensor_copy(
                        yT_bf[0:HD, hp, ic * 512:(ic + 1) * 512], pyA[0:HD, :])
                    nc.vector.tensor_copy(
                        yT_bf[HD:P, hp, ic * 512:(ic + 1) * 512], pyB[0:HD, :])
                    c = 4 * ic + hp
                    nc.vector.tensor_copy(
                        l_buf[0:1, c, :], pyA[HD:HD + 1, :])
                    nc.vector.tensor_copy(
                        l_buf[64:65, c, :], pyB[HD:HD + 1, :])
                    U_next(("norm", ic, hp), norm_unit, ic, hp)
                    pop_one()

                # all 4 norms must be emitted before proj units are queued
                pop_until(*[("norm", ic, h) for h in range(NHL // 2)])
                for tt in reversed(range(4 * ic, 4 * ic + 4)):
                    for oc in reversed(range(2)):
                        U_next(("proj", tt, oc), proj_unit, tt, oc)

            # tail: drain everything left
            while pop_one():
                pass
    if split_waits:
        split_multi_waits(nc)
    return nc


_PROGRAM = None


def _get_program():
    global _PROGRAM
    if _PROGRAM is None:
        _PROGRAM = build_program()
    return _PROGRAM


BF16 = ml_dtypes.bfloat16


def _make_in_maps(x, W_attn, b_attn, W_proj):
    x = np.asarray(x, dtype=np.float32)
    W_attn = np.asarray(W_attn, dtype=np.float32)
    b_attn = np.asarray(b_attn, dtype=np.float32)
    W_proj = np.asarray(W_proj, dtype=np.float32)
    in_maps = []
    for c in range(8):
        b, g = divmod(c, 2)
        sl = slice(CL * g, CL * (g + 1))
        wq = W_attn[:, 0:1024][:, sl] * SCALE
        wk = W_attn[:, 1024:2048][:, sl]
        wv = W_attn[:, 2048:3072][:, sl]
        bq = b_attn[0:1024][sl] * SCALE
        bk = b_attn[1024:2048][sl]
        bv = b_attn[2048:3072][sl]
        in_maps.append({
            "xT": np.ascontiguousarray(x[b].T.astype(BF16)),
            "wqk": np.ascontiguousarray(
                np.concatenate([wq, wk], axis=1).astype(BF16)),
            "wv": np.ascontiguousarray(wv.astype(BF16)),
            "bqk": np.ascontiguousarray(np.concatenate([bq, bk])),
            "bv": np.ascontiguousarray(bv),
            "wp": np.ascontiguousarray(W_proj[sl].astype(BF16)),
        })
    return in_maps


def kernel(x, W_attn, b_attn, W_proj, b_proj, _trace_dir=None):
    nc = _get_program()
    in_maps = _make_in_maps(x, W_attn, b_attn, W_proj)
    kwargs = {}
    if _trace_dir is not None:
        kwargs = dict(trace=True, tmpdir=_trace_dir)
    res = run_bass_kernel_spmd(nc, in_maps, core_ids=list(range(8)), **kwargs)
    b_proj = np.asarray(b_proj, dtype=np.float32)
    out = np.empty((4, T, COUT), dtype=np.float32)
    for b in range(4):
        out[b] = res.results[2 * b]["out"] + res.results[2 * b + 1]["out"] + b_proj
    if _trace_dir is not None:
        kernel._last_exec_time_ns = res.exec_time_ns
        kernel._last_results = res
    return out


# revision 16
# speedup vs baseline: 1.0302x; 1.0302x over previous
"""Causal self-attention kernel for 8 Trainium2 NeuronCores.

Sharding: core c -> (batch b = c // 2, head-group g = c % 2).
Each core computes attention for its batch over its 8 heads and a partial
output projection; the host sums the two head-group partials per batch and
adds b_proj.

v4: single fused pipeline.  The attention stream (S matmul pairs -> ACT exp
-> PV matmul pairs) is the backbone; all dense matmul work (qkv projection,
v tiles, output projection) is interleaved into it from a queue so the PE
never waits on the scalar engine.  x is pre-transposed and pre-cast to bf16
on the host (xT) and DMA'd directly into SBUF, as are all weights (one
batched strided DMA each, on the scalar HWDGE queue, concurrent with the
sync queue).  Causal masking zeroes the exp output on gpsimd
(affine_select).  1/l = exp(-ln(l)) on ACT - same table set as exp, no
table switches - computed per (i-chunk, head-pair) so normalization and
projection pipeline tightly behind attention.

Reference shapes: x [4, 2048, 1024], W_attn [1024, 3072], b_attn [3072],
W_proj [1024, 1024], b_proj [1024]; NH=16, HD=64.
"""

import ml_dtypes
import numpy as np

import bass_rust
import concourse.bass as bass
import concourse.mybir as mybir
import concourse.tile as tile
from concourse.bass_utils import run_bass_kernel_spmd

DT = mybir.dt
AF = mybir.ActivationFunctionType
ALU = mybir.AluOpType

P = 128
T = 2048          # sequence length
CIN = 1024        # input channels
CL = 512          # local channels (8 heads x 64)
NHL = 8           # local heads
HD = 64
KT = CIN // P     # 8 contraction tiles for qkv
TT = T // P       # 16 t-tiles
IC = T // 512     # 4 i-chunks of 512
COUT = 1024       # proj output channels
SCALE = 1.0 / 8.0  # 1/sqrt(HD), folded into wq/bq on host
LAG = 2


class PatchedTileContext(tile.TileContext):
    """Work around walrus's 1-sync-wait-per-Drain limit: split the final
    drain's waits across one Drain instruction per proc."""

    def _drain_and_barrier(self, tick_clock, wait_clock):
        ScopedClock = bass_rust.ScopedClock
        VectorClock = bass_rust.VectorClock
        ticks = eval(repr(tick_clock.global_clock).replace("VectorClock(", "").rstrip(")"))
        for p, t in [(p, t) for p, t in enumerate(ticks) if t > 0]:
            part = [0] * len(ticks)
            part[p] = t
            d = self.nc.sync.drain()
            wait_clock.add_sem_waits(d.ins, ScopedClock({None: VectorClock(part)}))
        self.nc.all_engine_barrier()
        popped = self.nc._tile_sem_poison_stack.pop()
        assert popped is self._sem_poison
        self.nc.clear_and_free_semaphores(list(self.sems.allocated().values()))
        self.nc.all_engine_barrier()


_MAX_WAITS = {}
_MAX_WAITS_DEFAULT = 1


def split_multi_waits(nc):
    for fn in nc.m.functions:
        for blk in fn.blocks:
            insts = blk.instructions
            out = []
            for inst in insts:
                si = getattr(inst, "sync_info", None)
                waits = list(si.on_wait) if si is not None and si.on_wait else []
                cap = _MAX_WAITS.get(str(inst.opcode), _MAX_WAITS_DEFAULT)
                if len(waits) > cap:
                    extra, keep = waits[:-cap], waits[-cap:]
                    for k, w in enumerate(extra):
                        nn = mybir.InstNoOp(name=f"{inst.name}-w{k}", ins=[], outs=[])
                        nn.engine = inst.engine
                        nn.sync_info = bass_rust.SyncInfo(on_wait=[w], on_update=[])
                        out.append(nn)
                    inst.sync_info = bass_rust.SyncInfo(
                        on_wait=keep, on_update=list(si.on_update or []))
                out.append(inst)
            blk.instructions = out


def build_program(split_waits=True):
    nc = bass.Bass()
    xT_d = nc.dram_tensor("xT", [CIN, T], DT.bfloat16, kind="ExternalInput")
    wqk_d = nc.dram_tensor("wqk", [CIN, 2 * CL], DT.bfloat16, kind="ExternalInput")
    wv_d = nc.dram_tensor("wv", [CIN, CL], DT.bfloat16, kind="ExternalInput")
    bqk_d = nc.dram_tensor("bqk", [2 * CL], DT.float32, kind="ExternalInput")
    bv_d = nc.dram_tensor("bv", [CL], DT.float32, kind="ExternalInput")
    wp_d = nc.dram_tensor("wp", [CL, COUT], DT.bfloat16, kind="ExternalInput")
    out_d = nc.dram_tensor("out", [T, COUT], DT.float32, kind="ExternalOutput")

    xT_r = xT_d.rearrange("(ko p) t -> p ko t", p=P)     # [128, 8, 2048]
    wqk_r = wqk_d.rearrange("(ko p) n -> p ko n", p=P)   # [128, 8, 1024]
    wv_r = wv_d.rearrange("(ko p) n -> p ko n", p=P)     # [128, 8, 512]
    wp_r = wp_d.rearrange("(ko p) n -> p ko n", p=P)     # [128, 4, 1024]
    out_r = out_d.rearrange("(tt p) c -> p tt c", p=P)   # [128, 16, 1024]

    with PatchedTileContext(nc) as tc:
        with (
            tc.tile_pool(name="const", bufs=1) as const,
            tc.tile_pool(name="big", bufs=1) as big,
            tc.tile_pool(name="pt", bufs=8) as pt_pool,
            tc.tile_pool(name="rr", bufs=2) as rr_pool,
            tc.tile_pool(name="outp", bufs=3) as outp,
            tc.tile_pool(name="ps_s", bufs=2, space="PSUM") as ps_s,
            tc.tile_pool(name="ps_y", bufs=2, space="PSUM") as ps_y,
            tc.tile_pool(name="ps_d", bufs=2, space="PSUM") as ps_d,
        ):
            # ---- constants ----
            ones1 = const.tile([65, P], DT.bfloat16, tag="ones1")
            nc.gpsimd.memset(ones1[:], 1.0)
            bqk_sb = const.tile([P, 2 * CL // P], DT.float32, tag="bqk")
            nc.sync.dma_start(bqk_sb[:], bqk_d.rearrange("(mt p) -> p mt", p=P))
            bv_sb = const.tile([P, NHL // 2], DT.float32, tag="bv")
            nc.sync.dma_start(
                bv_sb[:], bv_d.rearrange("(hp t p) -> (t p) hp", t=2, p=HD))

            # ---- persistent SBUF tensors ----
            xT_bf = big.tile([P, KT, T], DT.bfloat16, tag="xT")
            qkT_bf = big.tile([P, KT, T], DT.bfloat16, tag="qkT")
            v_sb = big.tile([P, TT, NHL, HD + 1], DT.bfloat16, tag="v_sb")
            yT_bf = big.tile([P, CL // P, T], DT.bfloat16, tag="yT")
            wqk_bf = big.tile([P, KT, 2 * CL], DT.bfloat16, tag="wqk_bf")
            wv_bf = big.tile([P, KT, CL], DT.bfloat16, tag="wv_bf")
            wp_bf = big.tile([P, CL // P, COUT], DT.bfloat16, tag="wp_bf")
            # l rows: head 0 at partition 0, head 1 at partition 64;
            # chunk 4*ic + hp
            l_buf = big.tile([65, 4 * IC, 512], DT.bfloat16, tag="l_buf")

            nc.gpsimd.memset(v_sb[:, :, :, HD], 1.0)

            # ---- head DMAs in need order on the sync queue; wp on the
            # scalar HWDGE queue (it only starts after the ACT table
            # load, ~10us in, which is still early enough for proj) ----
            nc.sync.dma_start(wqk_bf[:], wqk_r[:])
            nc.sync.dma_start(xT_bf[:, :, 0:512], xT_r[:, :, 0:512])
            nc.sync.dma_start(wv_bf[:], wv_r[:])
            nc.scalar.dma_start(wp_bf[:], wp_r[:])
            for nic in range(1, 4):
                nc.sync.dma_start(
                    xT_bf[:, :, nic * 512:(nic + 1) * 512],
                    xT_r[:, :, nic * 512:(nic + 1) * 512])

            # HAM warmup: keep the PE busy with throwaway matmuls while
            # the head DMAs land, so the clock gate is at 8/8 when the
            # real stream starts
            warm = ps_d.tile([P, 128], DT.float32, tag="d", name="warm")
            for _ in range(32):
                nc.tensor.matmul(
                    warm[:], ones1[0:65, :], ones1[0:65, 0:128],
                    start=True, stop=True)

            # ---- pipeline unit emitters ----
            def qkT_unit(nic, mi):
                pq = ps_d.tile([P, 512], DT.float32, tag="d", name=f"qk{nic}_{mi}")
                for ki in range(KT):
                    nc.tensor.matmul(
                        pq[:],
                        wqk_bf[:, ki, mi * P:(mi + 1) * P],
                        xT_bf[:, ki, nic * 512:(nic + 1) * 512],
                        start=(ki == 0), stop=(ki == KT - 1),
                    )
                nc.vector.tensor_scalar_add(
                    qkT_bf[:, mi, nic * 512:(nic + 1) * 512],
                    pq[:], bqk_sb[:, mi:mi + 1])

            def v_unit(tt):
                pv = ps_d.tile([P, 512], DT.float32, tag="d", name=f"v{tt}")
                for ki in range(KT):
                    nc.tensor.matmul(
                        pv[:],
                        xT_bf[:, ki, tt * P:(tt + 1) * P],
                        wv_bf[:, ki, :],
                        start=(ki == 0), stop=(ki == KT - 1),
                    )
                nc.vector.tensor_copy(
                    v_sb[:, tt, :, 0:HD],
                    pv[:].rearrange("p (h e) -> p h e", h=NHL),
                )

            def norm_unit(ic, hp):
                """yT[:, hp, ic-chunk] = z * exp(-ln(l)) + bv."""
                c = 4 * ic + hp
                rr1 = rr_pool.tile([65, 512], DT.float32, tag="rr1", name=f"ln{c}")
                nc.scalar.activation(rr1[:], l_buf[:, c, :], AF.Ln)
                rr2 = rr_pool.tile([65, 512], DT.bfloat16, tag="rr2", name=f"rx{c}")
                nc.scalar.activation(rr2[:], rr1[:], AF.Exp, scale=-1.0)
                pb = ps_d.tile([P, 512], DT.float32, tag="d", name=f"nb{c}")
                nc.tensor.matmul(
                    pb[0:HD, :], ones1[0:1, 0:HD], rr2[0:1, :],
                    start=True, stop=True)
                nc.tensor.matmul(
                    pb[HD:P, :], ones1[64:65, 0:HD], rr2[64:65, :],
                    start=True, stop=True, tile_position=(64, HD))
                ysl = yT_bf[:, hp, ic * 512:(ic + 1) * 512]
                nc.vector.tensor_mul(ysl, ysl, pb[:])
                nc.vector.tensor_scalar_add(ysl, ysl, bv_sb[:, hp:hp + 1])

            def proj_unit(tt, oc):
                pp = ps_d.tile([P, 512], DT.float32, tag="d", name=f"pj{tt}_{oc}")
                for ci in range(CL // P):
                    nc.tensor.matmul(
                        pp[:],
                        yT_bf[:, ci, tt * P:(tt + 1) * P],
                        wp_bf[:, ci, oc * 512:(oc + 1) * 512],
                        start=(ci == 0), stop=(ci == CL // P - 1),
                    )
                ot = outp.tile([P, 512], DT.float32, tag="ot", name=f"ot{tt}_{oc}")
                nc.vector.tensor_copy(ot[:], pp[:])
                nc.sync.dma_start(out_r[:, tt, oc * 512:(oc + 1) * 512], ot[:])

            # ---- dense work queue ----
            done_units = set()
            queue = []
            pop_idx = [0]

            def emit_unit(idx):
                key, fn = queue[idx]
                fn()
                done_units.add(key)

            def pop_one():
                while pop_idx[0] < len(queue):
                    i = pop_idx[0]
                    pop_idx[0] += 1
                    if queue[i][0] not in done_units:
                        emit_unit(i)
                        return True
                return False

            def pop_until(*keys):
                need = [k for k in keys if k not in done_units]
                for k in need:
                    assert any(q[0] == k for q in queue), f"missing unit {k}"
                while need:
                    if not pop_one():
                        raise AssertionError(f"queue exhausted, need {need}")
                    need = [k for k in keys if k not in done_units]

            def U(key, fn, *args):
                queue.append((key, (lambda f=fn, a=args: f(*a))))

            def U_next(key, fn, *args):
                queue.insert(pop_idx[0], (key, (lambda f=fn, a=args: f(*a))))

            # head compute
            qkT_unit(0, 0)
            qkT_unit(0, 4)
            done_units.update({("qkT", 0, 0), ("qkT", 0, 4)})

            U(("qkT", 0, 1), qkT_unit, 0, 1)
            U(("qkT", 0, 5), qkT_unit, 0, 5)
            for tt in range(4):
                U(("v", tt), v_unit, tt)
            U(("qkT", 0, 2), qkT_unit, 0, 2)
            U(("qkT", 0, 6), qkT_unit, 0, 6)
            U(("qkT", 0, 3), qkT_unit, 0, 3)
            U(("qkT", 0, 7), qkT_unit, 0, 7)
            for b in range(1, 4):
                for j, mi in enumerate([0, 4, 1, 5, 2, 6, 3, 7]):
                    U(("qkT", b, mi), qkT_unit, b, mi)
                    if j == 3:
                        for tt in range(4 * b, 4 * b + 4):
                            U(("v", tt), v_unit, tt)

            # paced pops per stripe, by i-chunk
            pace = {0: 2, 1: 2, 2: 2, 3: 4}

            # ---- attention ----
            for ic in range(IC):
                jt_max = 4 * ic + 3
                for hp in range(NHL // 2):
                    hA, hB = 2 * hp, 2 * hp + 1
                    qt, kt_i = hp, 4 + hp
                    pop_until(
                        ("qkT", ic, hp),
                        *[("qkT", n, 4 + hp) for n in range(ic + 1)],
                    )
                    budget = [pace[ic]]
                    pyA = ps_y.tile([HD + 1, 512], DT.float32, tag="y", name="pyA")
                    pyB = ps_y.tile([HD + 1, 512], DT.float32, tag="y", name="pyB")
                    pts = []

                    def emit_pv(jt):
                        pop_until(("v", jt))
                        d = jt - 4 * ic
                        off = 128 * d if d > 0 else 0
                        pt = pts[jt]
                        nc.tensor.matmul(
                            pyA[:, off:512], v_sb[:, jt, hA, :],
                            pt[:, off:512],
                            start=(jt == 0), stop=(jt == jt_max),
                            skip_group_check=True)
                        nc.tensor.matmul(
                            pyB[:, off:512], v_sb[:, jt, hB, :],
                            pt[:, 512 + off:1024],
                            start=(jt == 0), stop=(jt == jt_max),
                            skip_group_check=True)

                    for jt in range(jt_max + 1):
                        d = jt - 4 * ic
                        off = 128 * d if d > 0 else 0
                        ps = ps_s.tile([P, 1024], DT.float32, tag="s", name="st")
                        isl = slice(ic * 512 + off, (ic + 1) * 512)
                        nc.tensor.matmul(
                            ps[:, off:512],
                            qkT_bf[0:HD, kt_i, jt * P:(jt + 1) * P],
                            qkT_bf[0:HD, qt, isl],
                            start=True, stop=True)
                        nc.tensor.matmul(
                            ps[:, 512 + off:1024],
                            qkT_bf[HD:P, kt_i, jt * P:(jt + 1) * P],
                            qkT_bf[HD:P, qt, isl],
                            start=True, stop=True)
                        pt = pt_pool.tile([P, 1024], DT.bfloat16, tag="pt")
                        if d >= 0:
                            ps2 = ps[:].rearrange("p (g x) -> p g x", g=2)
                            pt2 = pt[:].rearrange("p (g x) -> p g x", g=2)
                            nc.scalar.activation(
                                pt2[:, :, off:512], ps2[:, :, off:512], AF.Exp)
                            nc.gpsimd.affine_select(
                                out=pt2[:, :, off:off + P],
                                in_=pt2[:, :, off:off + P],
                                compare_op=ALU.is_ge,
                                fill=0.0,
                                base=0,
                                pattern=[[0, 2], [1, P]],
                                channel_multiplier=-1,
                            )
                        else:
                            nc.scalar.activation(pt[:], ps[:], AF.Exp)
                        pts.append(pt)
                        if jt >= LAG:
                            emit_pv(jt - LAG)
                            if jt % 2 == 0 and budget[0] > 0:
                                if pop_one():
                                    budget[0] -= 1
                    for jt in range(max(0, jt_max + 1 - LAG), jt_max + 1):
                        emit_pv(jt)
                    # stash unnormalized z and the l rows
                    nc.vector.tensor_copy(
                        yT_bf[0:HD, hp, ic * 512:(ic + 1) * 512], pyA[0:HD, :])
                    nc.vector.tensor_copy(
                        yT_bf[HD:P, hp, ic * 512:(ic + 1) * 512], pyB[0:HD, :])
                    c = 4 * ic + hp
                    nc.vector.tensor_copy(
                        l_buf[0:1, c, :], pyA[HD:HD + 1, :])
                    nc.vector.tensor_copy(
                        l_buf[64:65, c, :], pyB[HD:HD + 1, :])
                    pop_one()
                    # insert norm AFTER the paced pop so it runs a stripe
                    # later (its ACT/DVE input chain needs time)
                    U_next(("norm", ic, hp), norm_unit, ic, hp)

                # proj units queued right after norm(ic, 3) (queue order
                # guarantees all 4 norms emit before any proj(ic))
                pos = pop_idx[0] + 1
                for tt in range(4 * ic, 4 * ic + 4):
                    for oc in range(2):
                        key = ("proj", tt, oc)
                        queue.insert(
                            pos, (key, (lambda t=tt, o=oc: proj_unit(t, o))))
                        pos += 1

            # tail: drain everything left
            while pop_one():
                pass
    if split_waits:
        split_multi_waits(nc)
    return nc


_PROGRAM = None


def _get_program():
    global _PROGRAM
    if _PROGRAM is None:
        _PROGRAM = build_program()
    return _PROGRAM


BF16 = ml_dtypes.bfloat16


def _make_in_maps(x, W_attn, b_attn, W_proj):
    x = np.asarray(x, dtype=np.float32)
    W_attn = np.asarray(W_attn, dtype=np.float32)
    b_attn = np.asarray(b_attn, dtype=np.float32)
    W_proj = np.asarray(W_proj, dtype=np.float32)
    in_maps = []
    for c in range(8):
        b, g = divmod(c, 2)
        sl = slice(CL * g, CL * (g + 1))
        wq = W_attn[:, 0:1024][:, sl] * SCALE
        wk = W_attn[:, 1024:2048][:, sl]
        wv = W_attn[:, 2048:3072][:, sl]
        bq = b_attn[0:1024][sl] * SCALE
        bk = b_attn[1024:2048][sl]
        bv = b_attn[2048:3072][sl]
        in_maps.append({
            "xT": np.ascontiguousarray(x[b].T.astype(BF16)),
            "wqk": np.ascontiguousarray(
                np.concatenate([wq, wk], axis=1).astype(BF16)),
            "wv": np.ascontiguousarray(wv.astype(BF16)),
            "bqk": np.ascontiguousarray(np.concatenate([bq, bk])),
            "bv": np.ascontiguousarray(bv),
            "wp": np.ascontiguousarray(W_proj[sl].astype(BF16)),
        })
    return in_maps


def kernel(x, W_attn, b_attn, W_proj, b_proj, _trace_dir=None):
    nc = _get_program()
    in_maps = _make_in_maps(x, W_attn, b_attn, W_proj)
    kwargs = {}
    if _trace_dir is not None:
        kwargs = dict(trace=True, tmpdir=_trace_dir)
    res = run_bass_kernel_spmd(nc, in_maps, core_ids=list(range(8)), **kwargs)
    b_proj = np.asarray(b_proj, dtype=np.float32)
    out = np.empty((4, T, COUT), dtype=np.float32)
    for b in range(4):
        out[b] = res.results[2 * b]["out"] + res.results[2 * b + 1]["out"] + b_proj
    if _trace_dir is not None:
        kernel._last_exec_time_ns = res.exec_time_ns
        kernel._last_results = res
    return out


# revision 22
# speedup vs baseline: 1.1498x; 1.1161x over previous
"""Causal self-attention kernel for 8 Trainium2 NeuronCores.

Sharding: core c -> (batch b = c // 2, head-group g = c % 2).
Each core computes attention for its batch over its 8 heads and a partial
output projection; the host sums the two head-group partials per batch and
adds b_proj.

v4: single fused pipeline.  The attention stream (S matmul pairs -> ACT exp
-> PV matmul pairs) is the backbone; all dense matmul work (qkv projection,
v tiles, output projection) is interleaved into it from a queue so the PE
never waits on the scalar engine.  x is pre-transposed and pre-cast to bf16
on the host (xT) and DMA'd directly into SBUF, as are all weights (one
batched strided DMA each, on the scalar HWDGE queue, concurrent with the
sync queue).  Causal masking zeroes the exp output on gpsimd
(affine_select).  1/l = exp(-ln(l)) on ACT - same table set as exp, no
table switches - computed per (i-chunk, head-pair) so normalization and
projection pipeline tightly behind attention.

Reference shapes: x [4, 2048, 1024], W_attn [1024, 3072], b_attn [3072],
W_proj [1024, 1024], b_proj [1024]; NH=16, HD=64.
"""

import ml_dtypes
import numpy as np

import bass_rust
import concourse.bass as bass
import concourse.mybir as mybir
import concourse.tile as tile
from concourse.bass_utils import run_bass_kernel_spmd

DT = mybir.dt
AF = mybir.ActivationFunctionType
ALU = mybir.AluOpType

P = 128
T = 2048          # sequence length
CIN = 1024        # input channels
CL = 512          # local channels (8 heads x 64)
NHL = 8           # local heads
HD = 64
KT = CIN // P     # 8 contraction tiles for qkv
TT = T // P       # 16 t-tiles
IC = T // 512     # 4 i-chunks of 512
COUT = 1024       # proj output channels
SCALE = 1.0 / 8.0  # 1/sqrt(HD), folded into wq/bq on host
LAG = 2


class PatchedTileContext(tile.TileContext):
    """Work around walrus's 1-sync-wait-per-Drain limit: split the final
    drain's waits across one Drain instruction per proc."""

    def _drain_and_barrier(self, tick_clock, wait_clock):
        ScopedClock = bass_rust.ScopedClock
        VectorClock = bass_rust.VectorClock
        ticks = eval(repr(tick_clock.global_clock).replace("VectorClock(", "").rstrip(")"))
        for p, t in [(p, t) for p, t in enumerate(ticks) if t > 0]:
            part = [0] * len(ticks)
            part[p] = t
            d = self.nc.sync.drain()
            wait_clock.add_sem_waits(d.ins, ScopedClock({None: VectorClock(part)}))
        self.nc.all_engine_barrier()
        popped = self.nc._tile_sem_poison_stack.pop()
        assert popped is self._sem_poison
        self.nc.clear_and_free_semaphores(list(self.sems.allocated().values()))
        self.nc.all_engine_barrier()


_MAX_WAITS = {}
_MAX_WAITS_DEFAULT = 1


def split_multi_waits(nc):
    for fn in nc.m.functions:
        for blk in fn.blocks:
            insts = blk.instructions
            out = []
            for inst in insts:
                si = getattr(inst, "sync_info", None)
                waits = list(si.on_wait) if si is not None and si.on_wait else []
                cap = _MAX_WAITS.get(str(inst.opcode), _MAX_WAITS_DEFAULT)
                if len(waits) > cap:
                    extra, keep = waits[:-cap], waits[-cap:]
                    for k, w in enumerate(extra):
                        nn = mybir.InstNoOp(name=f"{inst.name}-w{k}", ins=[], outs=[])
                        nn.engine = inst.engine
                        nn.sync_info = bass_rust.SyncInfo(on_wait=[w], on_update=[])
                        out.append(nn)
                    inst.sync_info = bass_rust.SyncInfo(
                        on_wait=keep, on_update=list(si.on_update or []))
                out.append(inst)
            blk.instructions = out


def build_program(split_waits=True):
    nc = bass.Bass()
    xT_d = nc.dram_tensor("xT", [CIN, T], DT.bfloat16, kind="ExternalInput")
    wqk_d = nc.dram_tensor("wqk", [CIN, 2 * CL], DT.bfloat16, kind="ExternalInput")
    wv_d = nc.dram_tensor("wv", [CIN, CL], DT.bfloat16, kind="ExternalInput")
    # host-packed biases: cols 0..7 = bqk m-tiles, 8..11 = bv head pairs
    bias_d = nc.dram_tensor("bias", [P, 12], DT.float32, kind="ExternalInput")
    wp_d = nc.dram_tensor("wp", [CL, COUT], DT.bfloat16, kind="ExternalInput")
    out_d = nc.dram_tensor("out", [T, COUT], DT.float32, kind="ExternalOutput")

    xT_r = xT_d.rearrange("(ko p) t -> p ko t", p=P)     # [128, 8, 2048]
    wqk_r = wqk_d.rearrange("(ko p) n -> p ko n", p=P)   # [128, 8, 1024]
    wv_r = wv_d.rearrange("(ko p) n -> p ko n", p=P)     # [128, 8, 512]
    wp_r = wp_d.rearrange("(ko p) n -> p ko n", p=P)     # [128, 4, 1024]
    out_r = out_d.rearrange("(tt p) c -> p tt c", p=P)   # [128, 16, 1024]

    with PatchedTileContext(nc) as tc:
        with (
            tc.tile_pool(name="const", bufs=1) as const,
            tc.tile_pool(name="big", bufs=1) as big,
            tc.tile_pool(name="pt", bufs=8) as pt_pool,
            tc.tile_pool(name="rr", bufs=2) as rr_pool,
            tc.tile_pool(name="outp", bufs=3) as outp,
            tc.tile_pool(name="ps_s", bufs=2, space="PSUM") as ps_s,
            tc.tile_pool(name="ps_y", bufs=2, space="PSUM") as ps_y,
            tc.tile_pool(name="ps_d", bufs=2, space="PSUM") as ps_d,
        ):
            # ---- constants ----
            ones1 = const.tile([65, P], DT.bfloat16, tag="ones1")
            nc.gpsimd.memset(ones1[:], 1.0)
            bias_sb = const.tile([P, 12], DT.float32, tag="bias")
            bqk_sb = bias_sb[:, 0:8]
            bv_sb = bias_sb[:, 8:12]

            # ---- persistent SBUF tensors ----
            xT_bf = big.tile([P, KT, T], DT.bfloat16, tag="xT")
            qkT_bf = big.tile([P, KT, T], DT.bfloat16, tag="qkT")
            v_sb = big.tile([P, TT, NHL, HD + 1], DT.bfloat16, tag="v_sb")
            yT_bf = big.tile([P, CL // P, T], DT.bfloat16, tag="yT")
            wqk_bf = big.tile([P, KT, 2 * CL], DT.bfloat16, tag="wqk_bf")
            wv_bf = big.tile([P, KT, CL], DT.bfloat16, tag="wv_bf")
            wp_bf = big.tile([P, CL // P, COUT], DT.bfloat16, tag="wp_bf")
            # l rows: head 0 at partition 0, head 1 at partition 64;
            # chunk 4*ic + hp
            l_buf = big.tile([65, 4 * IC, 512], DT.bfloat16, tag="l_buf")

            nc.gpsimd.memset(v_sb[:, :, :, HD], 1.0)

            # ---- head DMAs in need order on the sync queue; wp on the
            # scalar HWDGE queue (it only starts after the ACT table
            # load, ~10us in, which is still early enough for proj) ----
            nc.sync.dma_start(wqk_bf[:], wqk_r[:])
            nc.sync.dma_start(xT_bf[:, :, 0:512], xT_r[:, :, 0:512])
            nc.sync.dma_start(bias_sb, bias_d[:])
            nc.sync.dma_start(wv_bf[:], wv_r[:])
            nc.scalar.dma_start(wp_bf[:], wp_r[:])
            for nic in range(1, 4):
                nc.sync.dma_start(
                    xT_bf[:, :, nic * 512:(nic + 1) * 512],
                    xT_r[:, :, nic * 512:(nic + 1) * 512])

            # HAM warmup: keep the PE busy with throwaway matmuls while
            # the head DMAs land, so the clock gate is at 8/8 when the
            # real stream starts
            warm = ps_d.tile([P, 128], DT.float32, tag="d", name="warm")
            for _ in range(64):
                nc.tensor.matmul(
                    warm[:], ones1[0:65, :], ones1[0:65, 0:128],
                    start=True, stop=True)

            # ---- pipeline unit emitters ----
            def qkT_unit(nic, mi):
                pq = ps_d.tile([P, 512], DT.float32, tag="d", name=f"qk{nic}_{mi}")
                for ki in range(KT):
                    nc.tensor.matmul(
                        pq[:],
                        wqk_bf[:, ki, mi * P:(mi + 1) * P],
                        xT_bf[:, ki, nic * 512:(nic + 1) * 512],
                        start=(ki == 0), stop=(ki == KT - 1),
                    )
                nc.vector.tensor_scalar_add(
                    qkT_bf[:, mi, nic * 512:(nic + 1) * 512],
                    pq[:], bqk_sb[:, mi:mi + 1])

            def v_unit(tt):
                pv = ps_d.tile([P, 512], DT.float32, tag="d", name=f"v{tt}")
                for ki in range(KT):
                    nc.tensor.matmul(
                        pv[:],
                        xT_bf[:, ki, tt * P:(tt + 1) * P],
                        wv_bf[:, ki, :],
                        start=(ki == 0), stop=(ki == KT - 1),
                    )
                nc.vector.tensor_copy(
                    v_sb[:, tt, :, 0:HD],
                    pv[:].rearrange("p (h e) -> p h e", h=NHL),
                )

            def norm_unit(ic, hp):
                """yT[:, hp, ic-chunk] = z * exp(-ln(l)) + bv."""
                c = 4 * ic + hp
                rr1 = rr_pool.tile([65, 512], DT.float32, tag="rr1", name=f"ln{c}")
                nc.scalar.activation(rr1[:], l_buf[:, c, :], AF.Ln)
                rr2 = rr_pool.tile([65, 512], DT.bfloat16, tag="rr2", name=f"rx{c}")
                nc.scalar.activation(rr2[:], rr1[:], AF.Exp, scale=-1.0)
                pb = ps_d.tile([P, 512], DT.float32, tag="d", name=f"nb{c}")
                nc.tensor.matmul(
                    pb[0:HD, :], ones1[0:1, 0:HD], rr2[0:1, :],
                    start=True, stop=True)
                nc.tensor.matmul(
                    pb[HD:P, :], ones1[64:65, 0:HD], rr2[64:65, :],
                    start=True, stop=True, tile_position=(64, HD))
                ysl = yT_bf[:, hp, ic * 512:(ic + 1) * 512]
                nc.vector.tensor_mul(ysl, ysl, pb[:])
                nc.vector.tensor_scalar_add(ysl, ysl, bv_sb[:, hp:hp + 1])

            def proj_unit(tt, oc):
                pp = ps_d.tile([P, 512], DT.float32, tag="d", name=f"pj{tt}_{oc}")
                for ci in range(CL // P):
                    nc.tensor.matmul(
                        pp[:],
                        yT_bf[:, ci, tt * P:(tt + 1) * P],
                        wp_bf[:, ci, oc * 512:(oc + 1) * 512],
                        start=(ci == 0), stop=(ci == CL // P - 1),
                    )
                ot = outp.tile([P, 512], DT.float32, tag="ot", name=f"ot{tt}_{oc}")
                nc.vector.tensor_copy(ot[:], pp[:])
                nc.sync.dma_start(out_r[:, tt, oc * 512:(oc + 1) * 512], ot[:])

            # ---- dense work queue ----
            done_units = set()
            queue = []
            pop_idx = [0]

            def emit_unit(idx):
                key, fn = queue[idx]
                fn()
                done_units.add(key)

            def pop_one():
                while pop_idx[0] < len(queue):
                    i = pop_idx[0]
                    pop_idx[0] += 1
                    if queue[i][0] not in done_units:
                        emit_unit(i)
                        return True
                return False

            def pop_until(*keys):
                need = [k for k in keys if k not in done_units]
                for k in need:
                    assert any(q[0] == k for q in queue), f"missing unit {k}"
                while need:
                    if not pop_one():
                        raise AssertionError(f"queue exhausted, need {need}")
                    need = [k for k in keys if k not in done_units]

            def U(key, fn, *args):
                queue.append((key, (lambda f=fn, a=args: f(*a))))

            def U_next(key, fn, *args):
                queue.insert(pop_idx[0], (key, (lambda f=fn, a=args: f(*a))))

            # head compute
            qkT_unit(0, 0)
            qkT_unit(0, 4)
            done_units.update({("qkT", 0, 0), ("qkT", 0, 4)})

            U(("qkT", 0, 1), qkT_unit, 0, 1)
            U(("qkT", 0, 5), qkT_unit, 0, 5)
            for tt in range(4):
                U(("v", tt), v_unit, tt)
            U(("qkT", 0, 2), qkT_unit, 0, 2)
            U(("qkT", 0, 6), qkT_unit, 0, 6)
            U(("qkT", 0, 3), qkT_unit, 0, 3)
            U(("qkT", 0, 7), qkT_unit, 0, 7)
            for b in range(1, 4):
                for j, mi in enumerate([0, 4, 1, 5, 2, 6, 3, 7]):
                    U(("qkT", b, mi), qkT_unit, b, mi)
                    if j == 3:
                        for tt in range(4 * b, 4 * b + 4):
                            U(("v", tt), v_unit, tt)

            # paced pops per stripe, by i-chunk
            pace = {0: 2, 1: 3, 2: 3, 3: 5}

            # ---- attention ----
            for ic in range(IC):
                jt_max = 4 * ic + 3
                for hp in range(NHL // 2):
                    hA, hB = 2 * hp, 2 * hp + 1
                    qt, kt_i = hp, 4 + hp
                    pop_until(
                        ("qkT", ic, hp),
                        *[("qkT", n, 4 + hp) for n in range(ic + 1)],
                    )
                    budget = [pace[ic]]
                    pyA = ps_y.tile([HD + 1, 512], DT.float32, tag="y", name="pyA")
                    pyB = ps_y.tile([HD + 1, 512], DT.float32, tag="y", name="pyB")
                    pts = []

                    def emit_pv(jt):
                        pop_until(("v", jt))
                        d = jt - 4 * ic
                        off = 128 * d if d > 0 else 0
                        pt = pts[jt]
                        nc.tensor.matmul(
                            pyA[:, off:512], v_sb[:, jt, hA, :],
                            pt[:, off:512],
                            start=(jt == 0), stop=(jt == jt_max),
                            skip_group_check=True)
                        nc.tensor.matmul(
                            pyB[:, off:512], v_sb[:, jt, hB, :],
                            pt[:, 512 + off:1024],
                            start=(jt == 0), stop=(jt == jt_max),
                            skip_group_check=True)

                    for jt in range(jt_max + 1):
                        d = jt - 4 * ic
                        off = 128 * d if d > 0 else 0
                        ps = ps_s.tile([P, 1024], DT.float32, tag="s", name="st")
                        isl = slice(ic * 512 + off, (ic + 1) * 512)
                        nc.tensor.matmul(
                            ps[:, off:512],
                            qkT_bf[0:HD, kt_i, jt * P:(jt + 1) * P],
                            qkT_bf[0:HD, qt, isl],
                            start=True, stop=True)
                        nc.tensor.matmul(
                            ps[:, 512 + off:1024],
                            qkT_bf[HD:P, kt_i, jt * P:(jt + 1) * P],
                            qkT_bf[HD:P, qt, isl],
                            start=True, stop=True)
                        pt = pt_pool.tile([P, 1024], DT.bfloat16, tag="pt")
                        if d >= 0:
                            ps2 = ps[:].rearrange("p (g x) -> p g x", g=2)
                            pt2 = pt[:].rearrange("p (g x) -> p g x", g=2)
                            nc.scalar.activation(
                                pt2[:, :, off:512], ps2[:, :, off:512], AF.Exp)
                            nc.gpsimd.affine_select(
                                out=pt2[:, :, off:off + P],
                                in_=pt2[:, :, off:off + P],
                                compare_op=ALU.is_ge,
                                fill=0.0,
                                base=0,
                                pattern=[[0, 2], [1, P]],
                                channel_multiplier=-1,
                            )
                        else:
                            nc.scalar.activation(pt[:], ps[:], AF.Exp)
                        pts.append(pt)
                        if jt >= LAG:
                            emit_pv(jt - LAG)
                            if jt % 2 == 0 and budget[0] > 0:
                                if pop_one():
                                    budget[0] -= 1
                    for jt in range(max(0, jt_max + 1 - LAG), jt_max + 1):
                        emit_pv(jt)
                    # l rows first (they head the normalize chain), then
                    # the unnormalized z stash
                    c = 4 * ic + hp
                    nc.vector.tensor_copy(
                        l_buf[0:1, c, :], pyA[HD:HD + 1, :])
                    nc.vector.tensor_copy(
                        l_buf[64:65, c, :], pyB[HD:HD + 1, :])
                    nc.vector.tensor_copy(
                        yT_bf[0:HD, hp, ic * 512:(ic + 1) * 512], pyA[0:HD, :])
                    nc.vector.tensor_copy(
                        yT_bf[HD:P, hp, ic * 512:(ic + 1) * 512], pyB[0:HD, :])
                    pop_one()
                    # insert norm a few pops out so its ACT/DVE input
                    # chain (gathers -> ln -> exp) has time to complete
                    pos = min(pop_idx[0] + 3, len(queue))
                    key = ("norm", ic, hp)
                    queue.insert(pos, (key, (lambda i=ic, h=hp: norm_unit(i, h))))
                    if hp == NHL // 2 - 1:
                        # proj units directly after norm(ic, 3): queue
                        # order guarantees all 4 norms emit before proj(ic)
                        pos += 1
                        for tt in range(4 * ic, 4 * ic + 4):
                            for oc in range(2):
                                k2 = ("proj", tt, oc)
                                queue.insert(
                                    pos,
                                    (k2, (lambda t=tt, o=oc: proj_unit(t, o))))
                                pos += 1

            # tail: drain everything left
            while pop_one():
                pass
    if split_waits:
        split_multi_waits(nc)
    return nc


_PROGRAM = None


def _get_program():
    global _PROGRAM
    if _PROGRAM is None:
        _PROGRAM = build_program()
    return _PROGRAM


BF16 = ml_dtypes.bfloat16


def _make_in_maps(x, W_attn, b_attn, W_proj):
    x = np.asarray(x, dtype=np.float32)
    W_attn = np.asarray(W_attn, dtype=np.float32)
    b_attn = np.asarray(b_attn, dtype=np.float32)
    W_proj = np.asarray(W_proj, dtype=np.float32)
    in_maps = []
    for c in range(8):
        b, g = divmod(c, 2)
        sl = slice(CL * g, CL * (g + 1))
        wq = W_attn[:, 0:1024][:, sl] * SCALE
        wk = W_attn[:, 1024:2048][:, sl]
        wv = W_attn[:, 2048:3072][:, sl]
        bq = b_attn[0:1024][sl] * SCALE
        bk = b_attn[1024:2048][sl]
        bv = b_attn[2048:3072][sl]
        bqk = np.concatenate([bq, bk]).reshape(8, 128).T
        bvp = bv.reshape(4, 128).T
        in_maps.append({
            "xT": np.ascontiguousarray(x[b].T.astype(BF16)),
            "wqk": np.ascontiguousarray(
                np.concatenate([wq, wk], axis=1).astype(BF16)),
            "wv": np.ascontiguousarray(wv.astype(BF16)),
            "bias": np.ascontiguousarray(
                np.concatenate([bqk, bvp], axis=1).astype(np.float32)),
            "wp": np.ascontiguousarray(W_proj[sl].astype(BF16)),
        })
    return in_maps


def kernel(x, W_attn, b_attn, W_proj, b_proj, _trace_dir=None):
    nc = _get_program()
    in_maps = _make_in_maps(x, W_attn, b_attn, W_proj)
    kwargs = {}
    if _trace_dir is not None:
        kwargs = dict(trace=True, tmpdir=_trace_dir)
    res = run_bass_kernel_spmd(nc, in_maps, core_ids=list(range(8)), **kwargs)
    b_proj = np.asarray(b_proj, dtype=np.float32)
    out = np.empty((4, T, COUT), dtype=np.float32)
    for b in range(4):
        out[b] = res.results[2 * b]["out"] + res.results[2 * b + 1]["out"] + b_proj
    if _trace_dir is not None:
        kernel._last_exec_time_ns = res.exec_time_ns
        kernel._last_results = res
    return out


# revision 30
# speedup vs baseline: 1.1584x; 1.0075x over previous
"""Causal self-attention kernel for 8 Trainium2 NeuronCores.

Sharding: core c -> (batch b = c // 2, head-group g = c % 2).
Each core computes attention for its batch over its 8 heads and a partial
output projection; the host sums the two head-group partials per batch and
adds b_proj.

v4: single fused pipeline.  The attention stream (S matmul pairs -> ACT exp
-> PV matmul pairs) is the backbone; all dense matmul work (qkv projection,
v tiles, output projection) is interleaved into it from a queue so the PE
never waits on the scalar engine.  x is pre-transposed and pre-cast to bf16
on the host (xT) and DMA'd directly into SBUF, as are all weights (one
batched strided DMA each, on the scalar HWDGE queue, concurrent with the
sync queue).  Causal masking zeroes the exp output on gpsimd
(affine_select).  1/l = exp(-ln(l)) on ACT - same table set as exp, no
table switches - computed per (i-chunk, head-pair) so normalization and
projection pipeline tightly behind attention.

Reference shapes: x [4, 2048, 1024], W_attn [1024, 3072], b_attn [3072],
W_proj [1024, 1024], b_proj [1024]; NH=16, HD=64.
"""

import ml_dtypes
import numpy as np

import bass_rust
import concourse.bass as bass
import concourse.mybir as mybir
import concourse.tile as tile
from concourse.bass_utils import run_bass_kernel_spmd

DT = mybir.dt
AF = mybir.ActivationFunctionType
ALU = mybir.AluOpType

P = 128
T = 2048          # sequence length
CIN = 1024        # input channels
CL = 512          # local channels (8 heads x 64)
NHL = 8           # local heads
HD = 64
KT = CIN // P     # 8 contraction tiles for qkv
TT = T // P       # 16 t-tiles
IC = T // 512     # 4 i-chunks of 512
COUT = 1024       # proj output channels
SCALE = 1.0 / 8.0  # 1/sqrt(HD), folded into wq/bq on host
LAG = 2


class PatchedTileContext(tile.TileContext):
    """Work around walrus's 1-sync-wait-per-Drain limit: split the final
    drain's waits across one Drain instruction per proc."""

    def _drain_and_barrier(self, tick_clock, wait_clock):
        ScopedClock = bass_rust.ScopedClock
        VectorClock = bass_rust.VectorClock
        ticks = eval(repr(tick_clock.global_clock).replace("VectorClock(", "").rstrip(")"))
        for p, t in [(p, t) for p, t in enumerate(ticks) if t > 0]:
            part = [0] * len(ticks)
            part[p] = t
            d = self.nc.sync.drain()
            wait_clock.add_sem_waits(d.ins, ScopedClock({None: VectorClock(part)}))
        self.nc.all_engine_barrier()
        popped = self.nc._tile_sem_poison_stack.pop()
        assert popped is self._sem_poison
        self.nc.clear_and_free_semaphores(list(self.sems.allocated().values()))
        self.nc.all_engine_barrier()


_MAX_WAITS = {}
_MAX_WAITS_DEFAULT = 1


def split_multi_waits(nc):
    for fn in nc.m.functions:
        for blk in fn.blocks:
            insts = blk.instructions
            out = []
            for inst in insts:
                si = getattr(inst, "sync_info", None)
                waits = list(si.on_wait) if si is not None and si.on_wait else []
                cap = _MAX_WAITS.get(str(inst.opcode), _MAX_WAITS_DEFAULT)
                if len(waits) > cap:
                    extra, keep = waits[:-cap], waits[-cap:]
                    for k, w in enumerate(extra):
                        nn = mybir.InstNoOp(name=f"{inst.name}-w{k}", ins=[], outs=[])
                        nn.engine = inst.engine
                        nn.sync_info = bass_rust.SyncInfo(on_wait=[w], on_update=[])
                        out.append(nn)
                    inst.sync_info = bass_rust.SyncInfo(
                        on_wait=keep, on_update=list(si.on_update or []))
                out.append(inst)
            blk.instructions = out


def build_program(split_waits=True):
    nc = bass.Bass()
    # all inputs host-packed so every DMA is contiguous per partition
    # (1 descriptor per partition; descriptor generation is the head
    # bottleneck otherwise)
    xT_d = nc.dram_tensor("xT", [IC, P, KT, 512], DT.bfloat16, kind="ExternalInput")
    wqk_d = nc.dram_tensor("wqk", [8, P, KT, P], DT.bfloat16, kind="ExternalInput")
    wv_d = nc.dram_tensor("wv", [P, KT, CL], DT.bfloat16, kind="ExternalInput")
    # host-packed biases: cols 0..7 = bqk m-tiles, 8..11 = bv head pairs
    bias_d = nc.dram_tensor("bias", [P, 12], DT.float32, kind="ExternalInput")
    wp_d = nc.dram_tensor("wp", [P, CL // P, COUT], DT.bfloat16, kind="ExternalInput")
    out_d = nc.dram_tensor("out", [T, COUT], DT.float32, kind="ExternalOutput")

    out_r = out_d.rearrange("(tt p) c -> p tt c", p=P)   # [128, 16, 1024]

    with PatchedTileContext(nc) as tc:
        with (
            tc.tile_pool(name="const", bufs=1) as const,
            tc.tile_pool(name="big", bufs=1) as big,
            tc.tile_pool(name="pt", bufs=8) as pt_pool,
            tc.tile_pool(name="rr", bufs=2) as rr_pool,
            tc.tile_pool(name="outp", bufs=3) as outp,
            tc.tile_pool(name="ps_s", bufs=2, space="PSUM") as ps_s,
            tc.tile_pool(name="ps_y", bufs=2, space="PSUM") as ps_y,
            tc.tile_pool(name="ps_d", bufs=2, space="PSUM") as ps_d,
        ):
            # ---- constants ----
            ones1 = const.tile([65, P], DT.bfloat16, tag="ones1")
            nc.gpsimd.memset(ones1[:], 1.0)
            bias_sb = const.tile([P, 12], DT.float32, tag="bias")
            bqk_sb = bias_sb[:, 0:8]
            bv_sb = bias_sb[:, 8:12]

            # ---- persistent SBUF tensors ----
            xT_bf = big.tile([P, IC, KT, 512], DT.bfloat16, tag="xT")
            qkT_bf = big.tile([P, KT, T], DT.bfloat16, tag="qkT")
            v_sb = big.tile([P, TT, NHL, HD + 1], DT.bfloat16, tag="v_sb")
            yT_bf = big.tile([P, CL // P, T], DT.bfloat16, tag="yT")
            wqk_bf = big.tile([P, 8, KT, P], DT.bfloat16, tag="wqk_bf")
            wv_bf = big.tile([P, KT, CL], DT.bfloat16, tag="wv_bf")
            wp_bf = big.tile([P, CL // P, COUT], DT.bfloat16, tag="wp_bf")
            # l rows: head 0 at partition 0, head 1 at partition 64;
            # chunk 4*ic + hp
            l_buf = big.tile([65, 4 * IC, 512], DT.bfloat16, tag="l_buf")

            nc.gpsimd.memset(v_sb[:, :, :, HD], 1.0)

            # ---- head DMAs in need order on the sync queue; wp on the
            # scalar HWDGE queue (it only starts after the ACT table
            # load, ~10us in, which is still early enough for proj) ----
            def dma_wqk(mi):
                nc.sync.dma_start(wqk_bf[:, mi], wqk_d[mi])

            def dma_xT(nic):
                nc.sync.dma_start(xT_bf[:, nic], xT_d[nic])

            dma_wqk(0)
            dma_wqk(4)
            nc.sync.dma_start(bias_sb, bias_d[:])
            dma_xT(0)
            nc.sync.dma_start(wv_bf[:], wv_d[:])
            dma_wqk(1)
            dma_wqk(5)
            dma_xT(1)
            dma_wqk(2)
            dma_wqk(6)
            dma_wqk(3)
            dma_wqk(7)
            dma_xT(2)
            dma_xT(3)
            nc.scalar.dma_start(wp_bf[:], wp_d[:])

            # HAM warmup: keep the PE busy with throwaway matmuls while
            # the head DMAs land, so the clock gate is at 8/8 when the
            # real stream starts
            warm = ps_d.tile([P, 128], DT.float32, tag="d", name="warm")
            for _ in range(48):
                nc.tensor.matmul(
                    warm[:], ones1[0:65, :], ones1[0:65, 0:128],
                    start=True, stop=True)

            # ---- pipeline unit emitters ----
            def qkT_unit(nic, mi):
                pq = ps_d.tile([P, 512], DT.float32, tag="d", name=f"qk{nic}_{mi}")
                for ki in range(KT):
                    nc.tensor.matmul(
                        pq[:],
                        wqk_bf[:, mi, ki, :],
                        xT_bf[:, nic, ki, :],
                        start=(ki == 0), stop=(ki == KT - 1),
                    )
                nc.vector.tensor_scalar_add(
                    qkT_bf[:, mi, nic * 512:(nic + 1) * 512],
                    pq[:], bqk_sb[:, mi:mi + 1])

            def v_unit(tt):
                pv = ps_d.tile([P, 512], DT.float32, tag="d", name=f"v{tt}")
                lo = (tt % 4) * P
                for ki in range(KT):
                    nc.tensor.matmul(
                        pv[:],
                        xT_bf[:, tt // 4, ki, lo:lo + P],
                        wv_bf[:, ki, :],
                        start=(ki == 0), stop=(ki == KT - 1),
                    )
                nc.vector.tensor_copy(
                    v_sb[:, tt, :, 0:HD],
                    pv[:].rearrange("p (h e) -> p h e", h=NHL),
                )

            def norm_unit(ic, hp):
                """yT[:, hp, ic-chunk] = z * exp(-ln(l)) + bv."""
                c = 4 * ic + hp
                rr1 = rr_pool.tile([65, 512], DT.float32, tag="rr1", name=f"ln{c}")
                nc.scalar.activation(rr1[:], l_buf[:, c, :], AF.Ln)
                rr2 = rr_pool.tile([65, 512], DT.bfloat16, tag="rr2", name=f"rx{c}")
                nc.scalar.activation(rr2[:], rr1[:], AF.Exp, scale=-1.0)
                pb = ps_d.tile([P, 512], DT.float32, tag="d", name=f"nb{c}")
                nc.tensor.matmul(
                    pb[0:HD, :], ones1[0:1, 0:HD], rr2[0:1, :],
                    start=True, stop=True)
                nc.tensor.matmul(
                    pb[HD:P, :], ones1[64:65, 0:HD], rr2[64:65, :],
                    start=True, stop=True, tile_position=(64, HD))
                ysl = yT_bf[:, hp, ic * 512:(ic + 1) * 512]
                nc.vector.tensor_mul(ysl, ysl, pb[:])
                nc.vector.tensor_scalar_add(ysl, ysl, bv_sb[:, hp:hp + 1])

            def proj_unit(tt, oc):
                pp = ps_d.tile([P, 512], DT.float32, tag="d", name=f"pj{tt}_{oc}")
                for ci in range(CL // P):
                    nc.tensor.matmul(
                        pp[:],
                        yT_bf[:, ci, tt * P:(tt + 1) * P],
                        wp_bf[:, ci, oc * 512:(oc + 1) * 512],
                        start=(ci == 0), stop=(ci == CL // P - 1),
                    )
                ot = outp.tile([P, 512], DT.float32, tag="ot", name=f"ot{tt}_{oc}")
                # alternate copy engine so the tail doesn't serialize on
                # one engine's queue
                if (tt + oc) % 2 == 0:
                    nc.vector.tensor_copy(ot[:], pp[:])
                else:
                    nc.scalar.copy(ot[:], pp[:])
                nc.sync.dma_start(out_r[:, tt, oc * 512:(oc + 1) * 512], ot[:])

            # ---- dense work queue ----
            done_units = set()
            queue = []
            pop_idx = [0]

            def emit_unit(idx):
                key, fn = queue[idx]
                fn()
                done_units.add(key)

            def pop_one():
                while pop_idx[0] < len(queue):
                    i = pop_idx[0]
                    pop_idx[0] += 1
                    if queue[i][0] not in done_units:
                        emit_unit(i)
                        return True
                return False

            def pop_until(*keys):
                need = [k for k in keys if k not in done_units]
                for k in need:
                    assert any(q[0] == k for q in queue), f"missing unit {k}"
                while need:
                    if not pop_one():
                        raise AssertionError(f"queue exhausted, need {need}")
                    need = [k for k in keys if k not in done_units]

            def U(key, fn, *args):
                queue.append((key, (lambda f=fn, a=args: f(*a))))

            def U_next(key, fn, *args):
                queue.insert(pop_idx[0], (key, (lambda f=fn, a=args: f(*a))))

            # head compute
            qkT_unit(0, 0)
            qkT_unit(0, 4)
            done_units.update({("qkT", 0, 0), ("qkT", 0, 4)})

            U(("qkT", 0, 1), qkT_unit, 0, 1)
            U(("qkT", 0, 5), qkT_unit, 0, 5)
            for tt in range(4):
                U(("v", tt), v_unit, tt)
            U(("qkT", 0, 2), qkT_unit, 0, 2)
            U(("qkT", 0, 6), qkT_unit, 0, 6)
            U(("qkT", 0, 3), qkT_unit, 0, 3)
            U(("qkT", 0, 7), qkT_unit, 0, 7)
            for b in range(1, 4):
                for j, mi in enumerate([0, 4, 1, 5, 2, 6, 3, 7]):
                    U(("qkT", b, mi), qkT_unit, b, mi)
                    if j == 3:
                        for tt in range(4 * b, 4 * b + 4):
                            U(("v", tt), v_unit, tt)

            # paced pops per stripe, by (i-chunk, head-pair): hold some
            # units back from the last stripes so the drain has filler
            pace = {0: 2, 1: 3, 2: 3, 3: 6}
            pace_late = {(3, 2): 3, (3, 3): 3}

            # ---- attention ----
            for ic in range(IC):
                jt_max = 4 * ic + 3
                for hp in range(NHL // 2):
                    hA, hB = 2 * hp, 2 * hp + 1
                    qt, kt_i = hp, 4 + hp
                    pop_until(
                        ("qkT", ic, hp),
                        *[("qkT", n, 4 + hp) for n in range(ic + 1)],
                    )
                    budget = [pace_late.get((ic, hp), pace[ic])]
                    pyA = ps_y.tile([HD + 1, 512], DT.float32, tag="y", name="pyA")
                    pyB = ps_y.tile([HD + 1, 512], DT.float32, tag="y", name="pyB")
                    pts = []

                    def emit_pv(jt):
                        pop_until(("v", jt))
                        d = jt - 4 * ic
                        off = 128 * d if d > 0 else 0
                        pt = pts[jt]
                        nc.tensor.matmul(
                            pyA[:, off:512], v_sb[:, jt, hA, :],
                            pt[:, off:512],
                            start=(jt == 0), stop=(jt == jt_max),
                            skip_group_check=True)
                        nc.tensor.matmul(
                            pyB[:, off:512], v_sb[:, jt, hB, :],
                            pt[:, 512 + off:1024],
                            start=(jt == 0), stop=(jt == jt_max),
                            skip_group_check=True)

                    for jt in range(jt_max + 1):
                        d = jt - 4 * ic
                        off = 128 * d if d > 0 else 0
                        ps = ps_s.tile([P, 1024], DT.float32, tag="s", name="st")
                        isl = slice(ic * 512 + off, (ic + 1) * 512)
                        nc.tensor.matmul(
                            ps[:, off:512],
                            qkT_bf[0:HD, kt_i, jt * P:(jt + 1) * P],
                            qkT_bf[0:HD, qt, isl],
                            start=True, stop=True)
                        nc.tensor.matmul(
                            ps[:, 512 + off:1024],
                            qkT_bf[HD:P, kt_i, jt * P:(jt + 1) * P],
                            qkT_bf[HD:P, qt, isl],
                            start=True, stop=True)
                        pt = pt_pool.tile([P, 1024], DT.bfloat16, tag="pt")
                        if d >= 0:
                            ps2 = ps[:].rearrange("p (g x) -> p g x", g=2)
                            pt2 = pt[:].rearrange("p (g x) -> p g x", g=2)
                            nc.scalar.activation(
                                pt2[:, :, off:512], ps2[:, :, off:512], AF.Exp)
                            nc.gpsimd.affine_select(
                                out=pt2[:, :, off:off + P],
                                in_=pt2[:, :, off:off + P],
                                compare_op=ALU.is_ge,
                                fill=0.0,
                                base=0,
                                pattern=[[0, 2], [1, P]],
                                channel_multiplier=-1,
                            )
                        else:
                            nc.scalar.activation(pt[:], ps[:], AF.Exp)
                        pts.append(pt)
                        if jt >= LAG:
                            emit_pv(jt - LAG)
                            if jt % 2 == 0 and budget[0] > 0:
                                if pop_one():
                                    budget[0] -= 1
                    for jt in range(max(0, jt_max + 1 - LAG), jt_max + 1):
                        emit_pv(jt)
                    # l rows first (they head the normalize chain), then
                    # the unnormalized z stash
                    c = 4 * ic + hp
                    nc.vector.tensor_copy(
                        l_buf[0:1, c, :], pyA[HD:HD + 1, :])
                    nc.vector.tensor_copy(
                        l_buf[64:65, c, :], pyB[HD:HD + 1, :])
                    nc.vector.tensor_copy(
                        yT_bf[0:HD, hp, ic * 512:(ic + 1) * 512], pyA[0:HD, :])
                    nc.vector.tensor_copy(
                        yT_bf[HD:P, hp, ic * 512:(ic + 1) * 512], pyB[0:HD, :])
                    pop_one()
                    # insert norm a few pops out so its ACT/DVE input
                    # chain (gathers -> ln -> exp) has time to complete
                    pos = min(pop_idx[0] + 3, len(queue))
                    key = ("norm", ic, hp)
                    queue.insert(pos, (key, (lambda i=ic, h=hp: norm_unit(i, h))))
                    if hp == NHL // 2 - 1:
                        # proj units directly after norm(ic, 3): queue
                        # order guarantees all 4 norms emit before proj(ic)
                        pos += 1
                        for tt in range(4 * ic, 4 * ic + 4):
                            for oc in range(2):
                                k2 = ("proj", tt, oc)
                                queue.insert(
                                    pos,
                                    (k2, (lambda t=tt, o=oc: proj_unit(t, o))))
                                pos += 1

            # tail: drain everything left
            while pop_one():
                pass
    if split_waits:
        split_multi_waits(nc)
    return nc


_PROGRAM = None


def _get_program():
    global _PROGRAM
    if _PROGRAM is None:
        _PROGRAM = build_program()
    return _PROGRAM


BF16 = ml_dtypes.bfloat16


def _make_in_maps(x, W_attn, b_attn, W_proj):
    x = np.asarray(x, dtype=np.float32)
    W_attn = np.asarray(W_attn, dtype=np.float32)
    b_attn = np.asarray(b_attn, dtype=np.float32)
    W_proj = np.asarray(W_proj, dtype=np.float32)
    in_maps = []
    for c in range(8):
        b, g = divmod(c, 2)
        sl = slice(CL * g, CL * (g + 1))
        wq = W_attn[:, 0:1024][:, sl] * SCALE
        wk = W_attn[:, 1024:2048][:, sl]
        wv = W_attn[:, 2048:3072][:, sl]
        bq = b_attn[0:1024][sl] * SCALE
        bk = b_attn[1024:2048][sl]
        bv = b_attn[2048:3072][sl]
        bqk = np.concatenate([bq, bk]).reshape(8, 128).T
        bvp = bv.reshape(4, 128).T
        wqk = np.concatenate([wq, wk], axis=1)
        in_maps.append({
            # [nic, p, ko, t']: per-partition-contiguous t-chunks of x^T
            "xT": np.ascontiguousarray(
                x[b].reshape(4, 512, 8, 128).transpose(0, 3, 2, 1).astype(BF16)),
            # [mi, p, ko, c]: per-partition-contiguous m-tile chunks
            "wqk": np.ascontiguousarray(
                wqk.reshape(8, 128, 8, 128).transpose(2, 1, 0, 3).astype(BF16)),
            # [p, ko, c]
            "wv": np.ascontiguousarray(
                wv.reshape(8, 128, 512).transpose(1, 0, 2).astype(BF16)),
            "bias": np.ascontiguousarray(
                np.concatenate([bqk, bvp], axis=1).astype(np.float32)),
            # [p, ko, n]
            "wp": np.ascontiguousarray(
                W_proj[sl].reshape(4, 128, 1024).transpose(1, 0, 2).astype(BF16)),
        })
    return in_maps


def kernel(x, W_attn, b_attn, W_proj, b_proj, _trace_dir=None):
    nc = _get_program()
    in_maps = _make_in_maps(x, W_attn, b_attn, W_proj)
    kwargs = {}
    if _trace_dir is not None:
        kwargs = dict(trace=True, tmpdir=_trace_dir)
    res = run_bass_kernel_spmd(nc, in_maps, core_ids=list(range(8)), **kwargs)
    b_proj = np.asarray(b_proj, dtype=np.float32)
    out = np.empty((4, T, COUT), dtype=np.float32)
    for b in range(4):
        out[b] = res.results[2 * b]["out"] + res.results[2 * b + 1]["out"] + b_proj
    if _trace_dir is not None:
        kernel._last_exec_time_ns = res.exec_time_ns
        kernel._last_results = res
    return out
